# revision 41
# baseline (speedup 1.0000x reference)
"""Causal multi-head attention on 8 Trainium2 NeuronCores.

Problem: B=2, S=2048, D=1024, H=16, Dh=64 (fp32 in/out).
Sharding: core c handles batch b = c//4 and 4 heads [4g, 4g+4), g = c%4
(data parallel over batch x head-group tensor parallel). Each core returns
a partial attention output (its heads' z @ W_O); the host sums the 4 partials
per batch and adds the bias terms.

All matmul operands are bf16 (PSUM accumulation stays fp32): this enables
the PE fast-weight-load path (halves LDWEIGHTS), removes the f32r
small-free-dim penalty, and halves DMA traffic; rel err stays ~4.5e-3
against the fp32 reference.

On-core layout (everything transposed so no on-chip transposes are needed):
  x^T [d, s] comes pre-transposed (and pre-cast) from the host.
  Q^T, K^T [e, s] per head-pair (head A partitions 0-63, head B 64-127),
  produced by matmul(lhsT=W[d, e2], rhs=x^T[d, s]).
  V [s, e] natural, augmented with a ones column per head so the attention
  V-matmul also produces the softmax denominator.
  scores^T [k, q] = matmul(lhsT=K^T[e,k-tile], rhs=Q^T[e,q-tile]) - the K=64
  contraction auto-packs the two heads of a pair into disjoint PE row groups
  (concurrent on HW).
  E = exp(scores^T) with no max subtraction (logits are O(3), exp is safe);
  causal masking multiplies the diagonal blocks by a 0/1 mask slice; exp on
  high-diagonal tiles skips the fully-masked column range.
  z^T_aug [65, q] = matmul(lhsT=V_aug[k, 65], rhs=E[k, q]) accumulated over
  k-tiles; row 64 is the softmax denominator. Each finished z bank is
  drained to SBUF fast (den row + bf16 z copy) so the next pair's
  accumulation never waits on the recip/broadcast/mul normalization chain.
  out [s, d] = matmul(lhsT=z^T[e2, s-tile], rhs=W_O[e2, d]) accumulated over
  the two head pairs, staged to SBUF (DVE or, near the tail, ScalarE) and
  stored as bf16 partials the host upcasts and sums.

Scheduling: attention is emitted as a global software pipeline - the
scores/exp cursor runs LOOK=2 k-tile steps ahead of the z cursor, so the
PE's z matmuls always consume a 2-step-old exp and the ~1.1us exp latency
is never exposed, including across pair/block boundaries. Projection
chunks (next block's Q/K/V) and the previous block's output projection
weave into the z stream as PE filler; x tiles prefetch a full block ahead.

Scale 1/sqrt(Dh) is folded into W_Q/b_Q on the host. b_V's contribution
(sum_h b_V[h] @ W_O[h], constant per row since softmax weights sum to 1)
and b_O are added on the host.

Measured on 8 axon trn2 cores: 160.9us HW exec (baseline fp32r version:
215.6us), max rel err 4.5e-3 (gate 2e-2).
"""

import numpy as np

B, S, D, H, Dh = 2, 2048, 1024, 16, 64
NCORES = 8
CORES_PER_BATCH = 4
HPC = 4          # heads per core (= 2 pairs)
NPAIR = 2
DT_TILES = 8     # 1024 / 128
ST128 = 16       # s tiles of 128
SQ = 512         # q tile width
NQ4 = 4          # q tiles of 512

_BUILT = None


def _build():
    import concourse.bacc as bacc
    import concourse.mybir as mybir
    import concourse.tile as tile

    f32 = mybir.dt.float32
    bf16 = mybir.dt.bfloat16
    EXP = mybir.ActivationFunctionType.Exp

    nc = bacc.Bacc(None)

    xT = nc.dram_tensor("xT", [DT_TILES, 128, S], bf16, kind="ExternalInput")
    wq = nc.dram_tensor("wq", [NPAIR, 128, 1024], bf16, kind="ExternalInput")
    wk = nc.dram_tensor("wk", [NPAIR, 128, 1024], bf16, kind="ExternalInput")
    wv = nc.dram_tensor("wv", [128, 2048], bf16, kind="ExternalInput")
    wo = nc.dram_tensor("wo", [NPAIR, 128, 1024], bf16, kind="ExternalInput")
    bq = nc.dram_tensor("bq", [NPAIR, 128, 1], f32, kind="ExternalInput")
    bk = nc.dram_tensor("bk", [NPAIR, 128, 1], f32, kind="ExternalInput")
    maskd = nc.dram_tensor("maskd", [128, 896], bf16, kind="ExternalInput")
    out = nc.dram_tensor("out", [ST128, 128, 1024], bf16, kind="ExternalOutput")

    with tile.TileContext(nc) as tc:
        with (
            tc.tile_pool(name="const", bufs=1) as constp,
            tc.tile_pool(name="qkst", bufs=1) as qkstp,
            tc.tile_pool(name="xchunk", bufs=16) as xp,
            tc.tile_pool(name="work", bufs=3) as workp,
            tc.tile_pool(name="ps", bufs=1, space="PSUM") as ps,
        ):
            # ---- persistent constants / weights ----
            wq_sb = [constp.tile([128, 1024], bf16, tag=f"wq{p}", name=f"wq{p}")
                     for p in range(NPAIR)]
            wk_sb = [constp.tile([128, 1024], bf16, tag=f"wk{p}", name=f"wk{p}")
                     for p in range(NPAIR)]
            wv_sb = constp.tile([128, 2048], bf16, tag="wv", name="wv_sb")
            wo_sb = [constp.tile([128, 1024], bf16, tag=f"wo{p}", name=f"wo{p}")
                     for p in range(NPAIR)]
            bq_sb = [constp.tile([128, 1], f32, tag=f"bq{p}", name=f"bq{p}")
                     for p in range(NPAIR)]
            bk_sb = [constp.tile([128, 1], f32, tag=f"bk{p}", name=f"bk{p}")
                     for p in range(NPAIR)]
            mask_sb = constp.tile([128, 896], bf16, tag="mask", name="mask_sb")
            # all-ones regions of the mask reused as constants:
            # row 0 cols >=384 are 1.0; cols >=511 are 1.0 in every row
            ones_sb = mask_sb[0:1, 384:448]

            # pair-0 weights go out first - the very first matmul chain
            # needs wq[0] plus the x tiles, so don't queue x issues ahead
            nc.scalar.dma_start(wq_sb[0][:], wq[0])
            nc.scalar.dma_start(wk_sb[0][:], wk[0])
            for p in range(NPAIR):
                nc.gpsimd.dma_start(bq_sb[p][:], bq[p])
                nc.gpsimd.dma_start(bk_sb[p][:], bk[p])

            xc0 = []
            engs0 = (nc.sync, nc.scalar, nc.gpsimd, nc.sync,
                     nc.scalar, nc.gpsimd, nc.sync, nc.scalar)
            for t in range(DT_TILES):
                c0 = xp.tile([128, SQ], bf16, tag="x", name=f"x0_{t}")
                engs0[t].dma_start(c0[:], xT[t][:, 0:SQ])
                xc0.append(c0)

            nc.scalar.dma_start(wq_sb[1][:], wq[1])
            nc.scalar.dma_start(wk_sb[1][:], wk[1])
            nc.gpsimd.dma_start(wv_sb[:], wv[:])
            nc.gpsimd.dma_start(mask_sb[:], maskd[:])

            # dummy exp during the DMA head so the ~1.3us Exp table load
            # doesn't stall the first real activation
            warm = workp.tile([128, 1], f32, tag="warm", name="exp_warm")
            nc.scalar.activation(warm[:], bq_sb[0][:], EXP)

            # ---- persistent activations ----
            qt_sb = [qkstp.tile([128, S], bf16, tag=f"qt{p}", name=f"qt{p}")
                     for p in range(NPAIR)]
            kt_sb = [qkstp.tile([128, S], bf16, tag=f"kt{p}", name=f"kt{p}")
                     for p in range(NPAIR)]
            zt_sb = [qkstp.tile([128, S], bf16, tag=f"zt{p}", name=f"zt{p}")
                     for p in range(NPAIR)]
            # V, augmented: head h at cols [65h, 65h+64), ones col at 65h+64
            v_sb = [qkstp.tile([128, 260], bf16, tag=f"v{kt}", name=f"v{kt}")
                    for kt in range(ST128)]
            for kt in range(ST128):
                vo = v_sb[kt].rearrange("p (h c) -> p h c", c=65)
                nc.vector.tensor_copy(vo[:, :, 64], mask_sb[:, 511:515])

            # ---- phase 1 (per 512-wide s block): projections ----
            # returns a list of thunks so the caller can weave projection
            # chunks between attention steps (fills PE bubbles while exp runs)
            xcs = {0: xc0}

            def load_x(s4):
                sl = slice(s4 * SQ, (s4 + 1) * SQ)
                xc = xcs.setdefault(s4, [])
                for t in range(DT_TILES):
                    c = xp.tile([128, SQ], bf16, tag="x", name=f"x{s4}_{t}")
                    nc.sync.dma_start(c[:], xT[t][:, sl])
                    xc.append(c)

            def proj_chunks(s4):
                sl = slice(s4 * SQ, (s4 + 1) * SQ)
                xc = xcs[s4]

                def qk_chunk(p):
                    # single-bank Q and K tiles on the "v" tag: the scores
                    # s-tag rotation must never wait on these chunks' DVE
                    # bias-adds (v-tag waits land on weave filler instead)
                    q_ps = ps.tile([128, SQ], f32, tag="v", bufs=2,
                                   name=f"qps{s4}{p}")
                    for t in range(DT_TILES):
                        nc.tensor.matmul(q_ps[:], wq_sb[p][:, 128 * t:128 * t + 128], xc[t][:],
                                         start=(t == 0), stop=(t == DT_TILES - 1))
                    k_ps = ps.tile([128, SQ], f32, tag="v", bufs=2,
                                   name=f"kps{s4}{p}")
                    for t in range(DT_TILES):
                        nc.tensor.matmul(k_ps[:], wk_sb[p][:, 128 * t:128 * t + 128], xc[t][:],
                                         start=(t == 0), stop=(t == DT_TILES - 1))
                    nc.vector.tensor_scalar_add(qt_sb[p][:, sl], q_ps[:],
                                                bq_sb[p][:, 0:1])
                    nc.vector.tensor_scalar_add(kt_sb[p][:, sl], k_ps[:],
                                                bk_sb[p][:, 0:1])

                def v_chunk(j):
                    kt = 4 * s4 + j
                    v_ps = ps.tile([128, 256], f32, tag="v", bufs=2, name=f"vps{kt}")
                    for t in range(DT_TILES):
                        nc.tensor.matmul(v_ps[:],
                                         xc[t][:, j * 128:(j + 1) * 128],
                                         wv_sb[:, 256 * t:256 * t + 256],
                                         start=(t == 0), stop=(t == DT_TILES - 1))
                    vo = v_sb[kt].rearrange("p (h c) -> p h c", c=65)
                    vp = v_ps[:].rearrange("p (h c) -> p h c", c=64)
                    nc.vector.tensor_copy(vo[:, :, 0:64], vp)

                return ([("qk", lambda p=p: qk_chunk(p)) for p in range(NPAIR)]
                        + [("v", lambda j=j: v_chunk(j)) for j in range(4)])

            # ---- phase 3 (emitted interleaved): output projection ----
            def oproj_unit(st, half, otag, scalar_copy=False):
                ssl = slice(st * 128, (st + 1) * 128)
                dsl = slice(half * 512, (half + 1) * 512)
                o_ps = ps.tile([128, 512], f32, tag=otag, bufs=2,
                               name=f"ops{st}{half}")
                for p in range(NPAIR):
                    nc.tensor.matmul(o_ps[:], zt_sb[p][:, ssl],
                                     wo_sb[p][:, dsl],
                                     start=(p == 0), stop=(p == NPAIR - 1))
                o_sb = workp.tile([128, 512], bf16, tag="osb",
                                  name=f"osb{st}{half}")
                if scalar_copy:  # tail: ACT engine is idle by then
                    nc.scalar.copy(o_sb[:], o_ps[:])
                else:
                    nc.vector.tensor_copy(o_sb[:], o_ps[:])
                nc.sync.dma_start(out[st][:, dsl], o_sb[:])

            def emit_oproj(q4, sts=None, scalar_copy=False):
                for st in (range(4 * q4, 4 * q4 + 4) if sts is None else sts):
                    for half in range(2):
                        oproj_unit(st, half, "v", scalar_copy)

            # ---- phase 2 (per 512-wide q block): attention ----
            # ---- phase 2: attention, emitted as a global software
            # pipeline. The scores/exp cursor runs LOOK k-tile steps ahead
            # of the z cursor, so every z matmul consumes an exp issued two
            # steps earlier - the PE never waits out the exp latency, even
            # across pair and block boundaries. Projection chunks and
            # o-projection units weave into the z stream as PE filler.
            LOOK = 2
            flat = [(q4, p, kt)
                    for q4 in range(NQ4)
                    for p in range(NPAIR)
                    for kt in range(4 * q4 + 4)]
            etiles = {}
            zab = {}
            weave = []
            tail_hold = []
            wstate = {"per": 1, "step": 0}

            def enter_block(q4):
                # scores cursor first touches block q4: emit the NEXT
                # block's pair-0 qk chunk immediately (its x was prefetched
                # a block earlier); pair-1's qk, the v chunks and the
                # previous block's o-projection weave into the z stream as
                # PE filler. For the last block, two o-proj units are held
                # back to fill the tail normalization window.
                filler = []
                if q4 + 1 < NQ4:
                    if q4 + 2 < NQ4:
                        load_x(q4 + 2)
                    chunks = proj_chunks(q4 + 1)
                    qks = [c for tag, c in chunks if tag == "qk"]
                    qks[0]()
                    filler = [qks[1]] + [c for tag, c in chunks
                                         if tag == "v"]
                if q4 > 0:
                    sthalves = [(st, half)
                                for st in range(4 * (q4 - 1), 4 * (q4 - 1) + 4)
                                for half in range(2)]
                    if q4 == NQ4 - 1:
                        # hold the last four units for the tail window, with
                        # scalar-engine copies - the DVE queue is deep there
                        # and would pin their PSUM slots
                        tail_hold.extend(
                            lambda st=st, half=half:
                            oproj_unit(st, half, "v", True)
                            for st, half in sthalves[-4:])
                        sthalves = sthalves[:-4]
                    filler.extend(lambda st=st, half=half:
                                  oproj_unit(st, half, "v")
                                  for st, half in sthalves)
                weave.extend(filler)
                steps = NPAIR * (4 * q4 + 4)
                wstate["per"] = max(1, -(-steps // max(1, len(weave))))
                wstate["step"] = 0

            def scores_step(i):
                q4, p, kt = flat[i]
                if p == 0 and kt == 0:
                    enter_block(q4)
                q0 = q4 * SQ
                ksl = slice(kt * 128, (kt + 1) * 128)
                qsl = slice(q0, q0 + SQ)
                s_ps = ps.tile([128, 2 * SQ], f32, tag="s", bufs=2,
                               name=f"sps{q4}{p}{kt}")
                nc.tensor.matmul(s_ps[:, 0:SQ], kt_sb[p][0:64, ksl],
                                 qt_sb[p][0:64, qsl])
                nc.tensor.matmul(s_ps[:, SQ:2 * SQ], kt_sb[p][64:128, ksl],
                                 qt_sb[p][64:128, qsl])
                e = workp.tile([128, 2 * SQ], bf16, tag="e", bufs=6,
                               name=f"e{q4}{p}{kt}")
                d = kt * 128 - q0
                if d >= 256:
                    # high-diagonal tile: columns [0, d) are fully masked
                    # and never read - skip their exp
                    nc.scalar.activation(e[:, d:SQ], s_ps[:, d:SQ], EXP)
                    nc.scalar.activation(e[:, SQ + d:2 * SQ],
                                         s_ps[:, SQ + d:2 * SQ], EXP)
                else:
                    nc.scalar.activation(e[:], s_ps[:], EXP)
                etiles[i] = e

            def finish_pair(q4, p, za, zb):
                q0 = q4 * SQ
                qsl = slice(q0, q0 + SQ)
                last_pair = q4 == NQ4 - 1 and p == NPAIR - 1
                # drain each z bank to SBUF fast (den row + bf16 z copy) so
                # the next pair's accumulation isn't gated on the full
                # normalization chain; recip/broadcast/mul then run
                # entirely off-PSUM (den lands in a partition-0 tile - the
                # custom recip op and partition_broadcast need that). The
                # final pair skips the z drain: no later consumer of those
                # banks, and the shorter chain starts the tail o-proj sooner.
                rbs = []
                for sub, zps in ((0, za), (1, zb)):
                    den = workp.tile([1, SQ], f32, tag="den", bufs=4,
                                     name=f"den{q4}{p}{sub}")
                    nc.vector.tensor_copy(den[:], zps[64:65, :])
                    zu = None
                    if not last_pair:
                        zu = workp.tile([64, SQ], bf16, tag="zu", bufs=4,
                                        name=f"zu{q4}{p}{sub}")
                        nc.vector.tensor_copy(zu[:], zps[0:64, :])
                    rec = workp.tile([1, SQ], f32, tag="rec", bufs=4,
                                     name=f"rec{q4}{p}{sub}")
                    nc.vector.reciprocal_approx_fast(rec[:], den[:])
                    rb = workp.tile([64, SQ], f32, tag="rb", bufs=4,
                                    name=f"rb{q4}{p}{sub}")
                    nc.gpsimd.partition_broadcast(rb[:], rec[:])
                    rbs.append((sub, zps, zu, rb))
                if not last_pair:
                    for sub, zps, zu, rb in rbs:
                        nc.vector.tensor_mul(
                            zt_sb[p][64 * sub:64 * sub + 64, qsl],
                            zu[:], rb[:])
                else:
                    # tail: the held-back o-proj units (and leftover filler)
                    # fill the PE bubble while den/recip/broadcast run; then
                    # produce zt per 128-wide s-tile and emit that tile's
                    # output projection at once
                    while weave:
                        weave.pop(0)()
                    for u in tail_hold:
                        u()
                    for j in range(4):
                        jsl = slice(j * 128, (j + 1) * 128)
                        for sub, zps, zu, rb in rbs:
                            nc.vector.tensor_mul(
                                zt_sb[p][64 * sub:64 * sub + 64,
                                         q0 + j * 128:q0 + (j + 1) * 128],
                                zps[0:64, jsl], rb[:, jsl])
                        emit_oproj(q4, sts=[4 * q4 + j], scalar_copy=True)

            def z_step(i):
                q4, p, kt = flat[i]
                nk = 4 * q4 + 4
                q0 = q4 * SQ
                e = etiles.pop(i)
                if kt == 0:
                    zab[(q4, p)] = (
                        ps.tile([65, SQ], f32, tag="z", bufs=2,
                                name=f"za{q4}{p}"),
                        ps.tile([65, SQ], f32, tag="z", bufs=2,
                                name=f"zb{q4}{p}"))
                za, zb = zab[(q4, p)]
                d = kt * 128 - q0
                first = (kt == 0)
                for sub, zps in ((0, za), (1, zb)):
                    h = 2 * p + sub
                    vap = v_sb[kt][:, 65 * h:65 * h + 65]
                    ebase = sub * SQ
                    if d < 0:  # fully-allowed block
                        nc.tensor.matmul(
                            zps[:], vap, e[:, ebase:ebase + SQ],
                            start=first, stop=False,
                            skip_group_check=True)
                    else:
                        # columns [0, d) fully masked: skip.
                        # columns [d, d+128): mixed - mask-multiply.
                        em = workp.tile([128, 128], bf16, tag="em",
                                        bufs=6, name=f"em{q4}{p}{kt}{sub}")
                        nc.vector.tensor_mul(
                            em[:], e[:, ebase + d:ebase + d + 128],
                            mask_sb[:, 384:512])
                        nc.tensor.matmul(
                            zps[:, d:d + 128], vap, em[:],
                            start=first, stop=True,
                            skip_group_check=True)
                        # columns [d+128, 512): fully allowed.
                        if d + 128 < SQ:
                            nc.tensor.matmul(
                                zps[:, d + 128:SQ], vap,
                                e[:, ebase + d + 128:ebase + SQ],
                                start=first, stop=False,
                                skip_group_check=True)
                wstate["step"] += 1
                # extra pop right after each pair's first k-tile: the z
                # accumulation restart leaves the PE underfed there
                if weave and (wstate["step"] % wstate["per"] == 0 or kt == 0):
                    weave.pop(0)()
                if kt == nk - 1:
                    finish_pair(q4, p, za, zb)

            for _, c in proj_chunks(0):
                c()
            for p in range(NPAIR):  # W_O not needed until the first o-proj
                nc.gpsimd.dma_start(wo_sb[p][:], wo[p])
            load_x(1)  # streams in behind block 0's projection matmuls
            for i in range(len(flat) + LOOK):
                if i < len(flat):
                    scores_step(i)
                if i >= LOOK:
                    z_step(i - LOOK)

    nc.compile()
    return nc


def _get_built():
    global _BUILT
    if _BUILT is None:
        _BUILT = _build()
    return _BUILT


def _host_prep(x, W_Q, W_K, W_V, W_O, b_Q, b_K):
    """Build the 8 per-core input maps (matmul operands in bf16)."""
    import ml_dtypes
    bf16 = ml_dtypes.bfloat16
    scale = np.float32(1.0 / np.sqrt(Dh))
    mask = (np.arange(896)[None, :] >= (np.arange(128)[:, None] + 384)
            ).astype(bf16)
    in_maps = []
    for c in range(NCORES):
        b = c // CORES_PER_BATCH
        g = c % CORES_PER_BATCH
        hs = slice(HPC * g, HPC * g + HPC)
        xT_b = np.ascontiguousarray(x[b].T).reshape(
            DT_TILES, 128, S).astype(bf16)
        def pack_de(w):
            # [4 heads, D, Dh] -> pair-stacked [2, D, 128] -> [2, 128, 8*128]
            a = w.reshape(NPAIR, 2, D, Dh).transpose(0, 2, 1, 3).reshape(
                NPAIR, DT_TILES, 128, 128)
            return np.ascontiguousarray(a.transpose(0, 2, 1, 3)).reshape(
                NPAIR, 128, 1024).astype(bf16)

        wq_c = pack_de(W_Q[hs] * scale)
        wk_c = pack_de(W_K[hs])
        wv_c = np.ascontiguousarray(
            W_V[hs].transpose(1, 0, 2).reshape(DT_TILES, 128, HPC * Dh)
            .transpose(1, 0, 2)).reshape(128, 2048).astype(bf16)
        wo_c = np.ascontiguousarray(W_O[hs]).reshape(
            NPAIR, 128, 1024).astype(bf16)
        bq_c = np.ascontiguousarray(b_Q[hs] * scale).reshape(NPAIR, 128, 1)
        bk_c = np.ascontiguousarray(b_K[hs]).reshape(NPAIR, 128, 1)
        in_maps.append({
            "xT": xT_b, "wq": wq_c, "wk": wk_c, "wv": wv_c, "wo": wo_c,
            "bq": bq_c, "bk": bk_c, "maskd": mask,
        })
    return in_maps


def kernel(normalized_resid_pre, W_Q, W_K, W_V, W_O, b_Q, b_K, b_V, b_O,
           _want_profile=False):
    from concourse.bass_utils import run_bass_kernel_spmd

    x = np.asarray(normalized_resid_pre, np.float32)
    W_Q = np.asarray(W_Q, np.float32)
    W_K = np.asarray(W_K, np.float32)
    W_V = np.asarray(W_V, np.float32)
    W_O = np.asarray(W_O, np.float32)
    b_Q = np.asarray(b_Q, np.float32)
    b_K = np.asarray(b_K, np.float32)
    b_V = np.asarray(b_V, np.float32)
    b_O = np.asarray(b_O, np.float32)

    in_maps = _host_prep(x, W_Q, W_K, W_V, W_O, b_Q, b_K)
    nc = _get_built()
    kw = {}
    if _want_profile:
        kw = dict(trace=True)
    res = run_bass_kernel_spmd(nc, in_maps, list(range(NCORES)), **kw)

    # host-side unshard: sum the head-group partials per batch + bias terms
    b_eff = b_O + np.einsum("he,hed->d", b_V, W_O).astype(np.float32)
    attn_out = np.zeros((B, S, D), np.float32)
    for c in range(NCORES):
        b = c // CORES_PER_BATCH
        attn_out[b] += res.results[c]["out"].reshape(S, D).astype(np.float32)
    attn_out += b_eff[None, None, :]
    if _want_profile:
        return attn_out, res
    return attn_out



# revision 44
# speedup vs baseline: 1.0348x; 1.0348x over previous
"""Causal multi-head attention on 8 Trainium2 NeuronCores.

Problem: B=2, S=2048, D=1024, H=16, Dh=64 (fp32 in/out).
Sharding: core c handles batch b = c//4 and 4 heads [4g, 4g+4), g = c%4
(data parallel over batch x head-group tensor parallel). Each core returns
a partial attention output (its heads' z @ W_O); the host sums the 4 partials
per batch and adds the bias terms.

All matmul operands are bf16 (PSUM accumulation stays fp32): this enables
the PE fast-weight-load path (halves LDWEIGHTS), removes the f32r
small-free-dim penalty, and halves DMA traffic; rel err stays ~4.5e-3
against the fp32 reference.

On-core layout (everything transposed so no on-chip transposes are needed):
  x^T [d, s] comes pre-transposed (and pre-cast) from the host.
  Q^T, K^T [e, s] per head-pair (head A partitions 0-63, head B 64-127),
  produced by matmul(lhsT=W[d, e2], rhs=x^T[d, s]).
  V [s, e] natural, augmented with a ones column per head so the attention
  V-matmul also produces the softmax denominator.
  scores^T [k, q] = matmul(lhsT=K^T[e,k-tile], rhs=Q^T[e,q-tile]) - the K=64
  contraction auto-packs the two heads of a pair into disjoint PE row groups
  (concurrent on HW).
  E = exp(scores^T) with no max subtraction (logits are O(3), exp is safe);
  causal masking multiplies the diagonal blocks by a 0/1 mask slice; exp on
  high-diagonal tiles skips the fully-masked column range.
  z^T_aug [65, q] = matmul(lhsT=V_aug[k, 65], rhs=E[k, q]) accumulated over
  k-tiles; row 64 is the softmax denominator. Each finished z bank is
  drained to SBUF fast (den row + bf16 z copy) so the next pair's
  accumulation never waits on the recip/broadcast/mul normalization chain.
  out [s, d] = matmul(lhsT=z^T[e2, s-tile], rhs=W_O[e2, d]) accumulated over
  the two head pairs, staged to SBUF (DVE or, near the tail, ScalarE) and
  stored as bf16 partials the host upcasts and sums.

Scheduling: attention is emitted as a global software pipeline - the
scores/exp cursor runs LOOK=2 k-tile steps ahead of the z cursor, so the
PE's z matmuls always consume a 2-step-old exp and the ~1.1us exp latency
is never exposed, including across pair/block boundaries. Projection
chunks (next block's Q/K/V) and the previous block's output projection
weave into the z stream as PE filler; x tiles prefetch a full block ahead.

Scale 1/sqrt(Dh) is folded into W_Q/b_Q on the host. b_V's contribution
(sum_h b_V[h] @ W_O[h], constant per row since softmax weights sum to 1)
and b_O are added on the host.

Measured on 8 axon trn2 cores: 160.9us HW exec (baseline fp32r version:
215.6us), max rel err 4.5e-3 (gate 2e-2).
"""

import numpy as np

B, S, D, H, Dh = 2, 2048, 1024, 16, 64
NCORES = 8
CORES_PER_BATCH = 4
HPC = 4          # heads per core (= 2 pairs)
NPAIR = 2
DT_TILES = 8     # 1024 / 128
ST128 = 16       # s tiles of 128
SQ = 512         # q tile width
NQ4 = 4          # q tiles of 512

_BUILT = None


def _build():
    import concourse.bacc as bacc
    import concourse.mybir as mybir
    import concourse.tile as tile

    f32 = mybir.dt.float32
    bf16 = mybir.dt.bfloat16
    EXP = mybir.ActivationFunctionType.Exp

    nc = bacc.Bacc(None)

    xT = nc.dram_tensor("xT", [DT_TILES, 128, S], bf16, kind="ExternalInput")
    wq = nc.dram_tensor("wq", [NPAIR, 128, 1024], bf16, kind="ExternalInput")
    wk = nc.dram_tensor("wk", [NPAIR, 128, 1024], bf16, kind="ExternalInput")
    wv = nc.dram_tensor("wv", [128, 2048], bf16, kind="ExternalInput")
    wo = nc.dram_tensor("wo", [NPAIR, 128, 1024], bf16, kind="ExternalInput")
    bq = nc.dram_tensor("bq", [NPAIR, 128, 1], f32, kind="ExternalInput")
    bk = nc.dram_tensor("bk", [NPAIR, 128, 1], f32, kind="ExternalInput")
    maskd = nc.dram_tensor("maskd", [128, 896], bf16, kind="ExternalInput")
    out = nc.dram_tensor("out", [ST128, 128, 1024], bf16, kind="ExternalOutput")

    with tile.TileContext(nc) as tc:
        with (
            tc.tile_pool(name="const", bufs=1) as constp,
            tc.tile_pool(name="qkst", bufs=1) as qkstp,
            tc.tile_pool(name="xchunk", bufs=16) as xp,
            tc.tile_pool(name="work", bufs=3) as workp,
            tc.tile_pool(name="ps", bufs=1, space="PSUM") as ps,
        ):
            # ---- persistent constants / weights ----
            wq_sb = [constp.tile([128, 1024], bf16, tag=f"wq{p}", name=f"wq{p}")
                     for p in range(NPAIR)]
            wk_sb = [constp.tile([128, 1024], bf16, tag=f"wk{p}", name=f"wk{p}")
                     for p in range(NPAIR)]
            wv_sb = constp.tile([128, 2048], bf16, tag="wv", name="wv_sb")
            wo_sb = [constp.tile([128, 1024], bf16, tag=f"wo{p}", name=f"wo{p}")
                     for p in range(NPAIR)]
            bq_sb = [constp.tile([128, 1], f32, tag=f"bq{p}", name=f"bq{p}")
                     for p in range(NPAIR)]
            bk_sb = [constp.tile([128, 1], f32, tag=f"bk{p}", name=f"bk{p}")
                     for p in range(NPAIR)]
            mask_sb = constp.tile([128, 896], bf16, tag="mask", name="mask_sb")
            # all-ones regions of the mask reused as constants:
            # row 0 cols >=384 are 1.0; cols >=511 are 1.0 in every row
            ones_sb = mask_sb[0:1, 384:448]

            # pair-0 weights go out first - the very first matmul chain
            # needs wq[0] plus the x tiles, so don't queue x issues ahead
            nc.scalar.dma_start(wq_sb[0][:], wq[0])
            nc.scalar.dma_start(wk_sb[0][:], wk[0])
            for p in range(NPAIR):
                nc.gpsimd.dma_start(bq_sb[p][:], bq[p])
                nc.gpsimd.dma_start(bk_sb[p][:], bk[p])

            xc0 = []
            engs0 = (nc.sync, nc.scalar, nc.gpsimd, nc.sync,
                     nc.scalar, nc.gpsimd, nc.sync, nc.scalar)
            for t in range(DT_TILES):
                c0 = xp.tile([128, SQ], bf16, tag="x", name=f"x0_{t}")
                engs0[t].dma_start(c0[:], xT[t][:, 0:SQ])
                xc0.append(c0)

            nc.scalar.dma_start(wq_sb[1][:], wq[1])
            nc.scalar.dma_start(wk_sb[1][:], wk[1])
            nc.gpsimd.dma_start(wv_sb[:], wv[:])
            nc.gpsimd.dma_start(mask_sb[:], maskd[:])

            # dummy exp during the DMA head so the ~1.3us Exp table load
            # doesn't stall the first real activation
            warm = workp.tile([128, 1], f32, tag="warm", name="exp_warm")
            nc.scalar.activation(warm[:], bq_sb[0][:], EXP)

            # ---- persistent activations ----
            qt_sb = [qkstp.tile([128, S], bf16, tag=f"qt{p}", name=f"qt{p}")
                     for p in range(NPAIR)]
            kt_sb = [qkstp.tile([128, S], bf16, tag=f"kt{p}", name=f"kt{p}")
                     for p in range(NPAIR)]
            zt_sb = [qkstp.tile([128, S], bf16, tag=f"zt{p}", name=f"zt{p}")
                     for p in range(NPAIR)]
            # V, augmented: head h at cols [65h, 65h+64), ones col at 65h+64
            v_sb = [qkstp.tile([128, 260], bf16, tag=f"v{kt}", name=f"v{kt}")
                    for kt in range(ST128)]
            for kt in range(ST128):
                vo = v_sb[kt].rearrange("p (h c) -> p h c", c=65)
                nc.vector.tensor_copy(vo[:, :, 64], mask_sb[:, 511:515])

            # ---- phase 1 (per 512-wide s block): projections ----
            # returns a list of thunks so the caller can weave projection
            # chunks between attention steps (fills PE bubbles while exp runs)
            xcs = {0: xc0}

            def load_x(s4):
                sl = slice(s4 * SQ, (s4 + 1) * SQ)
                xc = xcs.setdefault(s4, [])
                for t in range(DT_TILES):
                    c = xp.tile([128, SQ], bf16, tag="x", name=f"x{s4}_{t}")
                    nc.sync.dma_start(c[:], xT[t][:, sl])
                    xc.append(c)

            def proj_chunks(s4):
                sl = slice(s4 * SQ, (s4 + 1) * SQ)
                xc = xcs[s4]

                def qk_chunk(p):
                    # single-bank Q and K tiles on the "v" tag: the scores
                    # s-tag rotation must never wait on these chunks' DVE
                    # bias-adds (v-tag waits land on weave filler instead)
                    q_ps = ps.tile([128, SQ], f32, tag="v", bufs=2,
                                   name=f"qps{s4}{p}")
                    for t in range(DT_TILES):
                        nc.tensor.matmul(q_ps[:], wq_sb[p][:, 128 * t:128 * t + 128], xc[t][:],
                                         start=(t == 0), stop=(t == DT_TILES - 1))
                    k_ps = ps.tile([128, SQ], f32, tag="v", bufs=2,
                                   name=f"kps{s4}{p}")
                    for t in range(DT_TILES):
                        nc.tensor.matmul(k_ps[:], wk_sb[p][:, 128 * t:128 * t + 128], xc[t][:],
                                         start=(t == 0), stop=(t == DT_TILES - 1))
                    nc.vector.tensor_scalar_add(qt_sb[p][:, sl], q_ps[:],
                                                bq_sb[p][:, 0:1])
                    nc.vector.tensor_scalar_add(kt_sb[p][:, sl], k_ps[:],
                                                bk_sb[p][:, 0:1])

                def v_chunk(j):
                    kt = 4 * s4 + j
                    v_ps = ps.tile([128, 256], f32, tag="v", bufs=2, name=f"vps{kt}")
                    for t in range(DT_TILES):
                        nc.tensor.matmul(v_ps[:],
                                         xc[t][:, j * 128:(j + 1) * 128],
                                         wv_sb[:, 256 * t:256 * t + 256],
                                         start=(t == 0), stop=(t == DT_TILES - 1))
                    vo = v_sb[kt].rearrange("p (h c) -> p h c", c=65)
                    vp = v_ps[:].rearrange("p (h c) -> p h c", c=64)
                    nc.vector.tensor_copy(vo[:, :, 0:64], vp)

                return ([("qk", lambda p=p: qk_chunk(p)) for p in range(NPAIR)]
                        + [("v", lambda j=j: v_chunk(j)) for j in range(4)])

            # ---- phase 3 (emitted interleaved): output projection ----
            def oproj_unit(st, half, otag, scalar_copy=False):
                ssl = slice(st * 128, (st + 1) * 128)
                dsl = slice(half * 512, (half + 1) * 512)
                o_ps = ps.tile([128, 512], f32, tag=otag, bufs=2,
                               name=f"ops{st}{half}")
                for p in range(NPAIR):
                    nc.tensor.matmul(o_ps[:], zt_sb[p][:, ssl],
                                     wo_sb[p][:, dsl],
                                     start=(p == 0), stop=(p == NPAIR - 1))
                o_sb = workp.tile([128, 512], bf16, tag="osb",
                                  name=f"osb{st}{half}")
                if scalar_copy:  # tail: ACT engine is idle by then
                    nc.scalar.copy(o_sb[:], o_ps[:])
                else:
                    nc.vector.tensor_copy(o_sb[:], o_ps[:])
                nc.sync.dma_start(out[st][:, dsl], o_sb[:])

            def emit_oproj(q4, sts=None, scalar_copy=False):
                for st in (range(4 * q4, 4 * q4 + 4) if sts is None else sts):
                    for half in range(2):
                        oproj_unit(st, half, "v", scalar_copy)

            # ---- phase 2 (per 512-wide q block): attention ----
            # ---- phase 2: attention, emitted as a global software
            # pipeline. The scores/exp cursor runs LOOK k-tile steps ahead
            # of the z cursor, so every z matmul consumes an exp issued two
            # steps earlier - the PE never waits out the exp latency, even
            # across pair and block boundaries. Projection chunks and
            # o-projection units weave into the z stream as PE filler.
            LOOK = 2
            flat = [(q4, p, kt)
                    for q4 in range(NQ4)
                    for p in range(NPAIR)
                    for kt in range(4 * q4 + 4)]
            etiles = {}
            zab = {}
            weave = []
            tail_hold = []
            pending = []  # deferred zt normalization muls (see finish_pair)
            wstate = {"per": 1, "step": 0}

            def enter_block(q4):
                # scores cursor first touches block q4: emit the NEXT
                # block's pair-0 qk chunk immediately (its x was prefetched
                # a block earlier); pair-1's qk, the v chunks and the
                # previous block's o-projection weave into the z stream as
                # PE filler. For the last block, two o-proj units are held
                # back to fill the tail normalization window.
                filler = []
                if q4 + 1 < NQ4:
                    if q4 + 2 < NQ4:
                        load_x(q4 + 2)
                    chunks = proj_chunks(q4 + 1)
                    qks = [c for tag, c in chunks if tag == "qk"]
                    qks[0]()
                    filler = [qks[1]] + [c for tag, c in chunks
                                         if tag == "v"]
                if q4 > 0:
                    sthalves = [(st, half)
                                for st in range(4 * (q4 - 1), 4 * (q4 - 1) + 4)
                                for half in range(2)]
                    if q4 == NQ4 - 1:
                        # hold the last four units for the tail window, with
                        # scalar-engine copies - the DVE queue is deep there
                        # and would pin their PSUM slots
                        tail_hold.extend(
                            lambda st=st, half=half:
                            oproj_unit(st, half, "v", True)
                            for st, half in sthalves[-4:])
                        sthalves = sthalves[:-4]
                    filler.extend(lambda st=st, half=half:
                                  oproj_unit(st, half, "v")
                                  for st, half in sthalves)
                weave.extend(filler)
                steps = NPAIR * (4 * q4 + 4)
                wstate["per"] = max(1, -(-steps // max(1, len(weave))))
                wstate["step"] = 0

            def scores_step(i):
                q4, p, kt = flat[i]
                if p == 0 and kt == 0:
                    enter_block(q4)
                q0 = q4 * SQ
                ksl = slice(kt * 128, (kt + 1) * 128)
                qsl = slice(q0, q0 + SQ)
                s_ps = ps.tile([128, 2 * SQ], f32, tag="s", bufs=2,
                               name=f"sps{q4}{p}{kt}")
                nc.tensor.matmul(s_ps[:, 0:SQ], kt_sb[p][0:64, ksl],
                                 qt_sb[p][0:64, qsl])
                nc.tensor.matmul(s_ps[:, SQ:2 * SQ], kt_sb[p][64:128, ksl],
                                 qt_sb[p][64:128, qsl])
                e = workp.tile([128, 2 * SQ], bf16, tag="e", bufs=6,
                               name=f"e{q4}{p}{kt}")
                d = kt * 128 - q0
                if d >= 256:
                    # high-diagonal tile: columns [0, d) are fully masked
                    # and never read - skip their exp
                    nc.scalar.activation(e[:, d:SQ], s_ps[:, d:SQ], EXP)
                    nc.scalar.activation(e[:, SQ + d:2 * SQ],
                                         s_ps[:, SQ + d:2 * SQ], EXP)
                else:
                    nc.scalar.activation(e[:], s_ps[:], EXP)
                etiles[i] = e

            def finish_pair(q4, p, za, zb):
                q0 = q4 * SQ
                qsl = slice(q0, q0 + SQ)
                last_pair = q4 == NQ4 - 1 and p == NPAIR - 1
                # drain each z bank to SBUF fast (den row + bf16 z copy) so
                # the next pair's accumulation isn't gated on the full
                # normalization chain; recip/broadcast/mul then run
                # entirely off-PSUM (den lands in a partition-0 tile - the
                # custom recip op and partition_broadcast need that). The
                # final pair skips the z drain: no later consumer of those
                # banks, and the shorter chain starts the tail o-proj sooner.
                rbs = []
                for sub, zps in ((0, za), (1, zb)):
                    den = workp.tile([1, SQ], f32, tag="den", bufs=4,
                                     name=f"den{q4}{p}{sub}")
                    nc.vector.tensor_copy(den[:], zps[64:65, :])
                    zu = None
                    if not last_pair:
                        zu = workp.tile([64, SQ], bf16, tag="zu", bufs=4,
                                        name=f"zu{q4}{p}{sub}")
                        nc.vector.tensor_copy(zu[:], zps[0:64, :])
                    rec = workp.tile([1, SQ], f32, tag="rec", bufs=4,
                                     name=f"rec{q4}{p}{sub}")
                    nc.vector.reciprocal_approx_fast(rec[:], den[:])
                    rb = workp.tile([64, SQ], f32, tag="rb", bufs=4,
                                    name=f"rb{q4}{p}{sub}")
                    nc.gpsimd.partition_broadcast(rb[:], rec[:])
                    rbs.append((sub, zps, zu, rb))
                if not last_pair:
                    # defer the zt muls into the next pair's stream: emitted
                    # here they sit at the DVE FIFO head gated on the fresh
                    # broadcast and block the em muls that feed the PE's
                    # diagonal z matmuls. zt is only read a block later
                    # (o-projection), and by the drain point rb is ready.
                    def muls(p=p, qsl=qsl, rbs=rbs):
                        for sub, zps, zu, rb in rbs:
                            nc.vector.tensor_mul(
                                zt_sb[p][64 * sub:64 * sub + 64, qsl],
                                zu[:], rb[:])
                    pending.append(muls)
                else:
                    # tail: the held-back o-proj units (and leftover filler)
                    # fill the PE bubble while den/recip/broadcast run; then
                    # produce zt per 128-wide s-tile and emit that tile's
                    # output projection at once
                    while weave:
                        weave.pop(0)()
                    for u in tail_hold:
                        u()
                    for j in range(4):
                        jsl = slice(j * 128, (j + 1) * 128)
                        for sub, zps, zu, rb in rbs:
                            nc.vector.tensor_mul(
                                zt_sb[p][64 * sub:64 * sub + 64,
                                         q0 + j * 128:q0 + (j + 1) * 128],
                                zps[0:64, jsl], rb[:, jsl])
                        emit_oproj(q4, sts=[4 * q4 + j], scalar_copy=True)

            def z_step(i):
                q4, p, kt = flat[i]
                nk = 4 * q4 + 4
                q0 = q4 * SQ
                # drain deferred norm muls: one filler-pop into the next
                # pair normally; immediately for the last block, whose first
                # woven o-proj unit consumes the previous block's zt
                if pending and kt == (0 if q4 == NQ4 - 1 else 1):
                    for t in pending:
                        t()
                    pending.clear()
                e = etiles.pop(i)
                if kt == 0:
                    zab[(q4, p)] = (
                        ps.tile([65, SQ], f32, tag="z", bufs=2,
                                name=f"za{q4}{p}"),
                        ps.tile([65, SQ], f32, tag="z", bufs=2,
                                name=f"zb{q4}{p}"))
                za, zb = zab[(q4, p)]
                d = kt * 128 - q0
                first = (kt == 0)
                for sub, zps in ((0, za), (1, zb)):
                    h = 2 * p + sub
                    vap = v_sb[kt][:, 65 * h:65 * h + 65]
                    ebase = sub * SQ
                    if d < 0:  # fully-allowed block
                        nc.tensor.matmul(
                            zps[:], vap, e[:, ebase:ebase + SQ],
                            start=first, stop=False,
                            skip_group_check=True)
                    else:
                        # columns [0, d) fully masked: skip.
                        # columns [d, d+128): mixed - mask-multiply.
                        em = workp.tile([128, 128], bf16, tag="em",
                                        bufs=6, name=f"em{q4}{p}{kt}{sub}")
                        nc.vector.tensor_mul(
                            em[:], e[:, ebase + d:ebase + d + 128],
                            mask_sb[:, 384:512])
                        nc.tensor.matmul(
                            zps[:, d:d + 128], vap, em[:],
                            start=first, stop=True,
                            skip_group_check=True)
                        # columns [d+128, 512): fully allowed.
                        if d + 128 < SQ:
                            nc.tensor.matmul(
                                zps[:, d + 128:SQ], vap,
                                e[:, ebase + d + 128:ebase + SQ],
                                start=first, stop=False,
                                skip_group_check=True)
                wstate["step"] += 1
                # extra pop right after each pair's first k-tile: the z
                # accumulation restart leaves the PE underfed there
                if weave and (wstate["step"] % wstate["per"] == 0 or kt == 0):
                    weave.pop(0)()
                if kt == nk - 1:
                    finish_pair(q4, p, za, zb)

            for _, c in proj_chunks(0):
                c()
            for p in range(NPAIR):  # W_O not needed until the first o-proj
                nc.gpsimd.dma_start(wo_sb[p][:], wo[p])
            load_x(1)  # streams in behind block 0's projection matmuls
            for i in range(len(flat) + LOOK):
                if i < len(flat):
                    scores_step(i)
                if i >= LOOK:
                    z_step(i - LOOK)

    nc.compile()
    return nc


def _get_built():
    global _BUILT
    if _BUILT is None:
        _BUILT = _build()
    return _BUILT


def _host_prep(x, W_Q, W_K, W_V, W_O, b_Q, b_K):
    """Build the 8 per-core input maps (matmul operands in bf16)."""
    import ml_dtypes
    bf16 = ml_dtypes.bfloat16
    scale = np.float32(1.0 / np.sqrt(Dh))
    mask = (np.arange(896)[None, :] >= (np.arange(128)[:, None] + 384)
            ).astype(bf16)
    in_maps = []
    for c in range(NCORES):
        b = c // CORES_PER_BATCH
        g = c % CORES_PER_BATCH
        hs = slice(HPC * g, HPC * g + HPC)
        xT_b = np.ascontiguousarray(x[b].T).reshape(
            DT_TILES, 128, S).astype(bf16)
        def pack_de(w):
            # [4 heads, D, Dh] -> pair-stacked [2, D, 128] -> [2, 128, 8*128]
            a = w.reshape(NPAIR, 2, D, Dh).transpose(0, 2, 1, 3).reshape(
                NPAIR, DT_TILES, 128, 128)
            return np.ascontiguousarray(a.transpose(0, 2, 1, 3)).reshape(
                NPAIR, 128, 1024).astype(bf16)

        wq_c = pack_de(W_Q[hs] * scale)
        wk_c = pack_de(W_K[hs])
        wv_c = np.ascontiguousarray(
            W_V[hs].transpose(1, 0, 2).reshape(DT_TILES, 128, HPC * Dh)
            .transpose(1, 0, 2)).reshape(128, 2048).astype(bf16)
        wo_c = np.ascontiguousarray(W_O[hs]).reshape(
            NPAIR, 128, 1024).astype(bf16)
        bq_c = np.ascontiguousarray(b_Q[hs] * scale).reshape(NPAIR, 128, 1)
        bk_c = np.ascontiguousarray(b_K[hs]).reshape(NPAIR, 128, 1)
        in_maps.append({
            "xT": xT_b, "wq": wq_c, "wk": wk_c, "wv": wv_c, "wo": wo_c,
            "bq": bq_c, "bk": bk_c, "maskd": mask,
        })
    return in_maps


def kernel(normalized_resid_pre, W_Q, W_K, W_V, W_O, b_Q, b_K, b_V, b_O,
           _want_profile=False):
    from concourse.bass_utils import run_bass_kernel_spmd

    x = np.asarray(normalized_resid_pre, np.float32)
    W_Q = np.asarray(W_Q, np.float32)
    W_K = np.asarray(W_K, np.float32)
    W_V = np.asarray(W_V, np.float32)
    W_O = np.asarray(W_O, np.float32)
    b_Q = np.asarray(b_Q, np.float32)
    b_K = np.asarray(b_K, np.float32)
    b_V = np.asarray(b_V, np.float32)
    b_O = np.asarray(b_O, np.float32)

    in_maps = _host_prep(x, W_Q, W_K, W_V, W_O, b_Q, b_K)
    nc = _get_built()
    kw = {}
    if _want_profile:
        kw = dict(trace=True)
    res = run_bass_kernel_spmd(nc, in_maps, list(range(NCORES)), **kw)

    # host-side unshard: sum the head-group partials per batch + bias terms
    b_eff = b_O + np.einsum("he,hed->d", b_V, W_O).astype(np.float32)
    attn_out = np.zeros((B, S, D), np.float32)
    for c in range(NCORES):
        b = c // CORES_PER_BATCH
        attn_out[b] += res.results[c]["out"].reshape(S, D).astype(np.float32)
    attn_out += b_eff[None, None, :]
    if _want_profile:
        return attn_out, res
    return attn_out

